# revision 25
# baseline (speedup 1.0000x reference)
"""Trainium2 Bass kernel for nn_EventADModel (2-layer event GRU + coord GRU + fusion MLP).

Strategy
--------
Pure data parallel across 8 NeuronCores: shard the valid (b,t) tracks into 8
shards.  Weights replicated.  Hidden/gate dim on SBUF partitions, tracks on
the free dim, NT=512 tracks per tile.

v7 (vs v6 baseline):
- All states kept NEGATED (s = -h) end-to-end; weight copies negated on host.
- First-step (h==0) gate activations merged: one sigmoid / one tanh per
  [128, 1024] 2-bank PSUM tile (bank-aligned matmul slices).
- Coord GRU batched 4 tiles per group, col-tiled onto all 128 partitions
  (32 hidden x 4 track-chunks): 4x fewer ACT cycles, 4x fewer PE passes.
- Frame-0 x-projection matmuls (PE rows 0:64) interleaved with frame-1
  x-partials (rows 64:128) so they pair in disjoint PE row-groups.
- Output computed transposed ([2, tracks] via one matmul with lhsT=W2.T);
  host transposes back.  b2 added on host.
"""

import os
import sys

for _p in ("/opt/trn_rl_repo",):
    if os.path.isdir(_p) and _p not in sys.path:
        sys.path.insert(0, _p)

import numpy as np

F16 = np.float16

B, F, T, X = 8192, 2, 30, 64
HE, HC = 256, 32
N_CORES = 8
N_TOT = B * T                 # 245760
NT = 512                      # tracks per main tile
G = 3 * HE                    # 768 gate rows

_CACHE = {}
LAST_RESULTS = None


def _pack_k(wT, m):
    """[k_tot, m] -> [128, (k_tot//128)*m] with K-chunks side by side."""
    kc = wT.shape[0] // 128
    return np.ascontiguousarray(
        wT.reshape(kc, 128, m).transpose(1, 0, 2).reshape(128, kc * m)
    )


def _build_program_v7(nc_tracks):
    """Zero-bias optimized build."""
    import concourse.bacc as bacc
    import concourse.mybir as mybir
    from concourse import tile

    dt = mybir.dt
    AF = mybir.ActivationFunctionType
    OP = mybir.AluOpType

    nc = bacc.Bacc("TRN2", target_bir_lowering=False, debug=False,
                   num_devices=N_CORES)

    xt_d = nc.dram_tensor("xt", [128, nc_tracks], dt.float16, kind="ExternalInput")
    ct_d = nc.dram_tensor("ct", [4, nc_tracks], dt.float16, kind="ExternalInput")
    out_d = nc.dram_tensor("out", [2, nc_tracks], dt.float32, kind="ExternalOutput")

    w0x_d = nc.dram_tensor("w0x", [128, 10 * 128], dt.float16,
                           kind="ExternalInput")
    w0hn_d = nc.dram_tensor("w0hn", [128, 2 * G], dt.float16, kind="ExternalInput")
    w1xn_d = nc.dram_tensor("w1xn", [128, 2 * G], dt.float16, kind="ExternalInput")
    w1hn_d = nc.dram_tensor("w1hn", [128, 2 * G], dt.float16, kind="ExternalInput")
    wc_d = nc.dram_tensor("wc", [4, 96], dt.float16, kind="ExternalInput")
    waen_d = nc.dram_tensor("waen", [128, 256], dt.float16, kind="ExternalInput")
    wacn4_d = nc.dram_tensor("wacn4", [128, 128], dt.float16, kind="ExternalInput")
    w2t_d = nc.dram_tensor("w2t", [128, 2], dt.float16, kind="ExternalInput")

    TILES = nc_tracks // NT

    with tile.TileContext(nc) as tc:
        with (
            tc.tile_pool(name="wpool", bufs=1) as wp,
            tc.tile_pool(name="xin", bufs=4) as xin,
            tc.tile_pool(name="gate", bufs=2) as gp,
            tc.tile_pool(name="state", bufs=4) as sp,
            tc.tile_pool(name="outp", bufs=2) as op_,
            tc.tile_pool(name="psA", bufs=3, space="PSUM") as psA,
            tc.tile_pool(name="psB", bufs=2, space="PSUM") as psB,
        ):
            w0x = wp.tile([128, 10 * 128], dt.float16, name="w0x_s")
            w0hn = wp.tile([128, 2 * G], dt.float16, name="w0hn_s")
            w1xn = wp.tile([128, 2 * G], dt.float16, name="w1xn_s")
            w1hn = wp.tile([128, 2 * G], dt.float16, name="w1hn_s")
            wc = wp.tile([4, 96], dt.float16, name="wc_s")
            waen = wp.tile([128, 256], dt.float16, name="waen_s")
            wacn4 = wp.tile([128, 128], dt.float16, name="wacn4_s")
            w2t = wp.tile([128, 2], dt.float16, name="w2t_s")
            for sb_t, dr in ((w0x, w0x_d), (w0hn, w0hn_d), (w1xn, w1xn_d),
                             (w1hn, w1hn_d), (wc, wc_d), (waen, waen_d),
                             (wacn4, wacn4_d), (w2t, w2t_d)):
                nc.sync.dma_start(sb_t[:], dr[:])

            def pa():  # 2-bank psum tile [128, 2*NT] f32
                return psA.tile([128, 2 * NT], dt.float32, name="pa", tag="A")

            def pb():  # 1-bank psum tile [128, NT] f32
                return psB.tile([128, NT], dt.float32, name="pb", tag="B")

            xts, nh0s, s02s, nh1s, s12s = {}, {}, {}, {}, {}
            bx = {}       # tile -> (r0b, r1b, z0b, z1b, gi0a, gi0b)
            nhc4s, outgs, gsizes = {}, {}, {}

            def coord_group(g):
                gsz = min(4, TILES - 4 * g)
                gsizes[g] = gsz
                ct4 = xin.tile([4, 4 * NT], dt.float16, name="ct4", tag="ct4")
                lo = 4 * g * NT
                nc.sync.dma_start(ct4[:, 0:gsz * NT], ct_d[:, lo:lo + gsz * NT])
                cz4 = pb()
                cn4 = pb()
                for j in range(gsz):
                    nc.tensor.matmul(cz4[32 * j:32 * j + 32, :], wc[:, 32:64],
                                     ct4[:, j * NT:(j + 1) * NT],
                                     start=True, stop=True,
                                     tile_position=(0, 32 * j))
                for j in range(gsz):
                    nc.tensor.matmul(cn4[32 * j:32 * j + 32, :], wc[:, 64:96],
                                     ct4[:, j * NT:(j + 1) * NT],
                                     start=True, stop=True,
                                     tile_position=(0, 32 * j))
                pp = 32 * gsz
                cz_s = gp.tile([128, NT], dt.float16, name="cz_s", tag="czs")
                cn_s = gp.tile([128, NT], dt.float16, name="cn_s", tag="cns")
                nc.scalar.activation(cz_s[0:pp, :], cz4[0:pp, :], AF.Sigmoid,
                                     scale=-1.0)
                nc.scalar.activation(cn_s[0:pp, :], cn4[0:pp, :], AF.Tanh)
                nhc4 = sp.tile([128, NT], dt.float16, name="nhc4", tag="nhc4",
                               bufs=3)
                nc.vector.tensor_mul(nhc4[0:pp, :], cz_s[0:pp, :],
                                     cn_s[0:pp, :])
                nhc4s[g] = nhc4

            def first_tail(zaps, naps, ztag, ntag, stag):
                """(1-z) via sigmoid(-x); h = (1-z)*tanh(n): POSITIVE state.
                zaps/naps: list of [128, NT] psum APs (gate halves)."""
                z_s = gp.tile([128, 2 * NT], dt.float16, name="z_s1", tag=ztag)
                n_s = gp.tile([128, 2 * NT], dt.float16, name="n_s1", tag=ntag)
                off = 0
                for ap in zaps:
                    wdt = ap.shape[-1]
                    nc.scalar.activation(z_s[:, off:off + wdt], ap,
                                         AF.Sigmoid, scale=-1.0)
                    off += wdt
                off = 0
                for ap in naps:
                    wdt = ap.shape[-1]
                    nc.scalar.activation(n_s[:, off:off + wdt], ap, AF.Tanh)
                    off += wdt
                s = sp.tile([128, 2 * NT], dt.float16, name="s_" + stag,
                            tag=stag)
                nc.vector.tensor_mul(s[:], z_s[:], n_s[:])
                return s

            def xmm(dst, chunk, xt_, start=True, stop=True):
                """64-contract x-projection MM.  Frame-0 chunks (0..3)
                live on PE rows 0:64, frame-1 chunks (4..9) on rows 64:128:
                disjoint row groups can run concurrently."""
                rows = slice(0, 64) if chunk < 4 else slice(64, 128)
                nc.tensor.matmul(dst, w0x[rows, chunk * 128:(chunk + 1) * 128],
                                 xt_[rows, :], start=start, stop=stop)

            def stage_ab(it):
                """A(it) frame-0 MMs paired with B(it-1) frame-1 x-MMs in
                the two banks of shared psA tiles: Tk = [A-half | B-half].
                Disjoint PE row groups (0:64 vs 64:128) + same ring slot =>
                the pair issues adjacently and runs concurrently."""
                a = it < TILES
                b = 1 <= it <= TILES
                T = [pa() for _ in range(4)]
                if a:
                    xt = xin.tile([128, NT], dt.float16, name="xt_t", tag="xt")
                    nc.sync.dma_start(xt[:], xt_d[:, it * NT:(it + 1) * NT])
                    xts[it] = xt
                if b:
                    xp = xts[it - 1]
                for k in range(4):
                    if a:
                        xmm(T[k][:, 0:NT], k, xt)
                    if b:
                        xmm(T[k][:, NT:2 * NT], 4 + k, xp, start=True,
                            stop=False)
                if a:
                    nh0s[it] = first_tail(
                        [T[0][:, 0:NT], T[1][:, 0:NT]],
                        [T[2][:, 0:NT], T[3][:, 0:NT]], "zsa", "nsa", "nh0")
                if b:
                    nh0 = nh0s[it - 1]
                    for k in range(4):
                        hmm_half(T[k][:, NT:2 * NT], w0hn, k, nh0,
                                 start=False)
                    gh0a, gh0b = pb(), pb()
                    hmm_half(gh0a[:], w0hn, 4, nh0, start=True)
                    hmm_half(gh0b[:], w0hn, 5, nh0, start=True)
                    gi0a, gi0b = pb(), pb()
                    xmm(gi0a[:], 8, xp)
                    xmm(gi0b[:], 9, xp)
                    bx[it - 1] = (T, gi0a, gi0b, gh0a, gh0b)

            def full_tail(raps, zaps, gia, gib, gha, ghb, s_prev, tg):
                r_s = gp.tile([128, 2 * NT], dt.float16, name="r_s", tag=tg + "r")
                z_s = gp.tile([128, 2 * NT], dt.float16, name="z_s", tag=tg + "z")
                off = 0
                for ap in raps:
                    wdt = ap.shape[-1]
                    nc.scalar.activation(r_s[:, off:off + wdt], ap, AF.Sigmoid)
                    off += wdt
                off = 0
                for ap in zaps:
                    wdt = ap.shape[-1]
                    nc.scalar.activation(z_s[:, off:off + wdt], ap, AF.Sigmoid)
                    off += wdt
                t = gp.tile([128, 2 * NT], dt.float16, name="t_", tag=tg + "t")
                nc.vector.tensor_mul(t[:, 0:NT], r_s[:, 0:NT], gha[:])
                nc.vector.tensor_mul(t[:, NT:2 * NT], r_s[:, NT:2 * NT], ghb[:])
                u = gp.tile([128, 2 * NT], dt.float16, name="u_", tag=tg + "u")
                nc.vector.tensor_add(u[:, 0:NT], t[:, 0:NT], gia[:])
                nc.vector.tensor_add(u[:, NT:2 * NT], t[:, NT:2 * NT], gib[:])
                n_s = gp.tile([128, 2 * NT], dt.float16, name="n_sf", tag=tg + "n")
                nc.scalar.activation(n_s[:], u[:], AF.Tanh)
                # h' = n - z*(n - h_prev)  (all tensor_tensor: 2x DVE mode)
                d = gp.tile([128, 2 * NT], dt.float16, name="d_", tag=tg + "d")
                nc.vector.tensor_sub(d[:], n_s[:], s_prev[:])
                e = gp.tile([128, 2 * NT], dt.float16, name="e_", tag=tg + "e")
                nc.vector.tensor_mul(e[:], z_s[:], d[:])
                s_new = sp.tile([128, 2 * NT], dt.float16, name="s_" + tg,
                                tag="s_" + tg)
                nc.vector.tensor_sub(s_new[:], n_s[:], e[:])
                return s_new

            def hmm_half(dst, w, j, rhs, start, stop=True):
                """contract-256 matmul pair accumulating into one bank slice.
                dst is a ready [128, NT] AP."""
                for kc in range(2):
                    nc.tensor.matmul(
                        dst, w[:, kc * G + j * 128:kc * G + (j + 1) * 128],
                        rhs[:, kc * NT:(kc + 1) * NT],
                        start=(start and kc == 0),
                        stop=(stop and kc == 1))

            def stage_bh(i):
                T, gi0a, gi0b, gh0a, gh0b = bx.pop(i)
                s02s[i] = full_tail(
                    [T[0][:, NT:2 * NT], T[1][:, NT:2 * NT]],
                    [T[2][:, NT:2 * NT], T[3][:, NT:2 * NT]],
                    gi0a, gi0b, gh0a, gh0b, nh0s[i], "b")

            def stage_c(i):
                nh0 = nh0s.pop(i)
                z1a = pa()
                n1a = pa()
                for dst, j0 in ((z1a, 2), (n1a, 4)):
                    for c in range(2):
                        j = j0 + c
                        for kc in range(2):
                            nc.tensor.matmul(
                                dst[:, c * NT:(c + 1) * NT],
                                w1xn[:, kc * G + j * 128:kc * G + (j + 1) * 128],
                                nh0[:, kc * NT:(kc + 1) * NT],
                                start=(kc == 0), stop=(kc == 1))
                nh1s[i] = first_tail([z1a[:]], [n1a[:]],
                                     "zsc", "nsc", "nh1")

            def stage_d(i):
                s02 = s02s.pop(i)
                nh1 = nh1s.pop(i)
                rD = pa()
                zD = pa()
                for dst, j0 in ((rD, 0), (zD, 2)):
                    for c in range(2):
                        sl = slice(c * NT, (c + 1) * NT)
                        hmm_half(dst[:, sl], w1xn, j0 + c, s02, start=True,
                                 stop=False)
                        hmm_half(dst[:, sl], w1hn, j0 + c, nh1, start=False)
                gh1a, gh1b = pb(), pb()
                hmm_half(gh1a[:], w1hn, 4, nh1, start=True)
                hmm_half(gh1b[:], w1hn, 5, nh1, start=True)
                gi1a, gi1b = pb(), pb()
                hmm_half(gi1a[:], w1xn, 4, s02, start=True)
                hmm_half(gi1b[:], w1xn, 5, s02, start=True)
                s12s[i] = full_tail([rD[:]], [zD[:]],
                                    gi1a, gi1b, gh1a, gh1b, nh1, "d")

            def stage_e(i):
                g, j = i // 4, i % 4
                s12 = s12s.pop(i)
                nhc4 = nhc4s[g]
                hid_ps = pb()
                nc.tensor.matmul(hid_ps[:], waen[:, 0:128], s12[:, 0:NT],
                                 start=True, stop=False)
                nc.tensor.matmul(hid_ps[:], waen[:, 128:256], s12[:, NT:2 * NT],
                                 start=False, stop=False)
                nc.tensor.matmul(hid_ps[:], wacn4[32 * j:32 * j + 32, :],
                                 nhc4[32 * j:32 * j + 32, :],
                                 start=False, stop=True,
                                 tile_position=(32 * j, 0))
                hid = gp.tile([128, NT], dt.float16, name="hid", tag="hid")
                nc.vector.tensor_scalar(hid[:], hid_ps[:], 0.0, None, OP.max)
                w2o = pb()
                nc.tensor.matmul(w2o[0:2, :], w2t[:], hid[:],
                                 start=True, stop=True)
                if j == 0:
                    outgs[g] = op_.tile([2, 4 * NT], dt.float32, name="outg",
                                        tag="outg")
                outg = outgs[g]
                nc.scalar.copy(outg[:, j * NT:(j + 1) * NT], w2o[0:2, :])
                if j == gsizes[g] - 1:
                    lo = 4 * g * NT
                    nc.sync.dma_start(out_d[:, lo:lo + gsizes[g] * NT],
                                      outg[:, 0:gsizes[g] * NT])
                    del outgs[g]

            for it in range(TILES + 5):
                if it % 4 == 0 and it < TILES:
                    coord_group(it // 4)
                if it <= TILES:
                    stage_ab(it)
                if 1 <= it <= TILES:
                    stage_bh(it - 1)
                if 2 <= it <= TILES + 1:
                    stage_c(it - 2)
                if 3 <= it <= TILES + 2:
                    stage_d(it - 3)
                if 4 <= it <= TILES + 3:
                    stage_e(it - 4)
                if it >= 1:
                    xts.pop(it - 1, None)

    nc.compile()
    return nc


def _prep_host(inputs):
    f32 = np.float32
    bf = np.asarray(inputs["batch_features"], dtype=f32)
    coords = np.asarray(inputs["coords"], dtype=f32)
    w = {k: np.asarray(inputs[k], dtype=f32) for k in inputs
         if k not in ("batch_features", "coords", "valid_mask")}

    XT = bf.transpose(1, 3, 0, 2).reshape(128, N_TOT)
    CT = coords.transpose(2, 0, 1).reshape(4, N_TOT)

    W1a, W1b = w["W1"][:, :128], w["W1"][:, 128:]
    waen = _pack_k(np.ascontiguousarray((W1a @ w["We"]).T), 128)
    wacn = np.ascontiguousarray((W1b @ w["Wc"]).T)  # [32, 128]
    wacn4 = np.concatenate([wacn] * 4, axis=0)       # [128, 128]

    zero_bias = all(
        not np.any(w[k]) for k in
        ("bih0", "bhh0", "bih1", "bhh1", "bihC", "bhhC", "be", "bc", "b1"))

    w0xT = np.ascontiguousarray(w["Wih0"].T)  # [64, 768]
    # zero-padded full-contract x-projection chunks:
    # 0..3 = frame0 z0,z1,n0,n1 (rows 0:64); 4..9 = frame1 r..n (rows 64:128)
    w0xz = np.zeros((128, 10 * 128), np.float32)
    for k in range(4):
        w0xz[0:64, k * 128:(k + 1) * 128] = w0xT[:, 256 + k * 128:
                                                 384 + k * 128]
    for k in range(6):
        w0xz[64:128, (4 + k) * 128:(5 + k) * 128] = w0xT[:, k * 128:
                                                         (k + 1) * 128]
    w1xT = _pack_k(np.ascontiguousarray(w["Wih1"].T), G)
    wd = {
        "w0x": w0xz.astype(F16),
        "w0hn": _pack_k(np.ascontiguousarray(w["Whh0"].T), G).astype(F16),
        "w1xn": w1xT.astype(F16),
        "w1hn": _pack_k(np.ascontiguousarray(w["Whh1"].T), G).astype(F16),
        "wc": np.ascontiguousarray(w["WihC"].T).astype(F16),
        "waen": waen.astype(F16),
        "wacn4": np.ascontiguousarray(wacn4).astype(F16),
        "w2t": np.ascontiguousarray(w["W2"].T).astype(F16),
    }
    return XT, CT, wd, zero_bias, w["b2"]


def _ensure_profile_hook_stub():
    try:
        import antenv.axon_hooks  # noqa: F401
    except Exception:
        import types
        try:
            import antenv
        except Exception:
            return
        mod = types.ModuleType("antenv.axon_hooks")
        mod.get_axon_ntff_profile_hook = lambda: None
        mod.set_axon_ntff_profile_hook = lambda h: None
        sys.modules["antenv.axon_hooks"] = mod
        antenv.axon_hooks = mod


def kernel(**inputs):
    global LAST_RESULTS
    _ensure_profile_hook_stub()
    from concourse.bass_utils import run_bass_kernel_spmd

    XT, CT, wd, zero_bias, b2 = _prep_host(inputs)
    if not zero_bias:
        raise NotImplementedError("v7 kernel requires zero GRU/MLP biases")

    mask = np.asarray(inputs["valid_mask"]).reshape(-1)
    idx = np.flatnonzero(mask)
    nv = idx.size
    if nv == 0:
        return np.zeros((B, T, 2), np.float32)
    chunk = N_CORES * NT
    n_pad = ((nv + chunk - 1) // chunk) * chunk
    nc_tracks = n_pad // N_CORES

    xt_c = np.zeros((128, n_pad), F16)
    xt_c[:, :nv] = XT.take(idx, axis=1).astype(F16)
    ct_c = np.zeros((4, n_pad), F16)
    ct_c[:, :nv] = CT.take(idx, axis=1).astype(F16)

    key = ("v7", nc_tracks)
    if key not in _CACHE:
        _CACHE[key] = _build_program_v7(nc_tracks)
    nc = _CACHE[key]

    in_maps = []
    for c in range(N_CORES):
        sl = slice(c * nc_tracks, (c + 1) * nc_tracks)
        m = dict(wd)
        m["xt"] = np.ascontiguousarray(xt_c[:, sl])
        m["ct"] = np.ascontiguousarray(ct_c[:, sl])
        in_maps.append(m)

    res = run_bass_kernel_spmd(nc, in_maps, list(range(N_CORES)))
    LAST_RESULTS = res

    outv = np.concatenate([res.results[c]["out"] for c in range(N_CORES)],
                          axis=1)[:, :nv].T
    outv = outv + b2[None, :].astype(np.float32)
    out = np.zeros((N_TOT, 2), np.float32)
    out[idx] = outv
    return out.reshape(B, T, 2)
